# revision 17
# baseline (speedup 1.0000x reference)
"""DeepHam GCN-scan kernel for Trainium2 (8 NeuronCores, replicated SPMD).

Reference computation (N=512 nodes, D=32 features, E=8192 edges):
  - dense normalized adjacency with self loops:  Ahat = D^-1/2 (A+I) D^-1/2
  - 512 sequential steps; each step:
      v = tanh(Ahat @ (v @ W_l) + b_l)   for l = 1,2,3
      probs = relu(v @ Wm1 + bm1) @ Wm2 + bm2
      out[t] = v[argmax(probs)]
  - the carried state v does NOT depend on the argmax selection.

Device strategy (single-core program, replicated on all 8 cores; the scan
is inherently sequential so cross-core sharding would only add per-layer
collective latency):
  - state kept transposed vT [32, 512] in SBUF; Ahat^T resident in SBUF.
  - all matmuls run in float32r (12-bit-mantissa round-to-nearest operands,
    single PE pass). Persistent GCN weights use the exact split
    W_r = round12(W), W_c = W - W_r. Constraints learned on HW:
      * bf16/8-bit state FAILS correctness (argmax flips on ~1e-4 prob
        gaps -> rel err 3e-2 > 2e-2 gate); 12-bit state is safe (1.3e-4).
      * fp32r matmuls only support tile_position (0,0): any col/row-group
        packing trips the s3d3_mm_valid_dst_partition ISA check or hangs,
        so the PE array width cannot be subdivided.
  - per layer: 4 matmuls (lhsT = vT 128-col slice, rhs = [W_r | W_c])
    produce chunked [v@W_r | v@W_c] [128, 64] x4; a strided DVE reduce
    sums the pairs into ts [128,128] fp32r; the A-multiply streams
    Ahat^T in two column halves (8 matmuls N=256) so tanh(half 0)
    overlaps the PE streaming half 1; tanh is split the same way and
    feeds the next layer's W-matmuls chunk-by-chunk.
  - HAM management: the PE clock gate only opens (1.2 -> 2.4 GHz) after
    ~3.4us of near-continuous matmul activity and re-closes during idle
    windows. Engines execute their instruction stream IN PROGRAM ORDER
    (semaphore-gated), so dummy filler matmuls (scratch PSUM output) are
    emitted exactly where the PE would otherwise idle: after the W-group
    (covers the DVE pair-reduce) and after the A-group (covers tanh).
  - readout: probs^T [1,512] via two matmuls + relu; one-hot by compare
    with the row max; chosen row extracted with an outer-product matmul
    (bf16, exact for 0/1) + multiply + reduce. Bitwise prob ties are
    handled by also emitting the tie count; the host divides by it.
"""

import os
import numpy as np

N, D = 512, 32
KC = 4  # 512 / 128 contraction chunks
NH = N // 2
N_STEPS = int(os.environ.get("DH_STEPS", str(N)))
MM_DT = os.environ.get("DH_MM_DT", "float32r")  # float32 | float32r
N_WARM = int(os.environ.get("DH_WARM", "24"))  # HAM warmup matmuls
N_FILL_A = int(os.environ.get("DH_FILL_A", "3"))  # fillers after A-group
N_FILL_W = int(os.environ.get("DH_FILL_W", "2"))  # fillers after W-group
_CACHE = {}


def _build(n_steps, mm_dt_name):
    import concourse.bacc as bacc
    import concourse.mybir as mybir
    from concourse.tile import TileContext

    dt = mybir.dt
    f32 = dt.float32
    bf16 = dt.bfloat16
    mdt = getattr(dt, mm_dt_name)
    AF = mybir.ActivationFunctionType
    ALU = mybir.AluOpType
    AX = mybir.AxisListType

    nc = bacc.Bacc(None, target_bir_lowering=False)

    atT = nc.dram_tensor("atT", [128, KC * N], mdt, kind="ExternalInput")
    vT0 = nc.dram_tensor("vT0", [D, N], mdt, kind="ExternalInput")
    # per layer [W_r | W_c]: W_r = round12(W) exact under fp32r, W_c = W - W_r
    wg = nc.dram_tensor("wg", [D, 3 * 2 * D], mdt, kind="ExternalInput")
    bg = nc.dram_tensor("bg", [D, 3], f32, kind="ExternalInput")
    wm1 = nc.dram_tensor("wm1", [D, D], mdt, kind="ExternalInput")
    bm1 = nc.dram_tensor("bm1", [D, 1], f32, kind="ExternalInput")
    wm2 = nc.dram_tensor("wm2", [D, 1], mdt, kind="ExternalInput")
    ones = nc.dram_tensor("ones", [1, D], f32, kind="ExternalInput")
    outT = nc.dram_tensor("outT", [D, n_steps], f32, kind="ExternalOutput")
    ct = nc.dram_tensor("ct", [1, n_steps], f32, kind="ExternalOutput")

    with TileContext(nc) as tc:
        with (
            tc.tile_pool(name="const", bufs=1) as cpool,
            tc.tile_pool(name="vstate", bufs=3) as vpool,
            tc.tile_pool(name="tchunk", bufs=2) as tpool,
            tc.tile_pool(name="ro", bufs=2) as ropool,
            tc.tile_pool(name="pt", bufs=2, space="PSUM") as ppt,
            tc.tile_pool(name="pu", bufs=2, space="PSUM") as ppu,
            tc.tile_pool(name="pro", bufs=3, space="PSUM") as ppro,
            tc.tile_pool(name="pw", bufs=1, space="PSUM") as ppw,
        ):
            # ---- constants into SBUF ----
            at_sb = cpool.tile([128, KC * N], mdt)
            nc.sync.dma_start(at_sb[:], atT[:, :])
            wg_sb = cpool.tile([D, 3 * 2 * D], mdt)
            nc.sync.dma_start(wg_sb[:], wg[:, :])
            bg_sb = cpool.tile([D, 3], f32)
            nc.sync.dma_start(bg_sb[:], bg[:, :])
            wm1_sb = cpool.tile([D, D], mdt)
            nc.sync.dma_start(wm1_sb[:], wm1[:, :])
            bm1_sb = cpool.tile([D, 1], f32)
            nc.sync.dma_start(bm1_sb[:], bm1[:, :])
            wm2_sb = cpool.tile([D, 1], mdt)
            nc.sync.dma_start(wm2_sb[:], wm2[:, :])
            ones_f = cpool.tile([1, D], f32)
            nc.sync.dma_start(ones_f[:], ones[:, :])
            ones_b = cpool.tile([1, D], bf16)
            nc.vector.tensor_copy(ones_b[:], ones_f[:])

            outT_sb = cpool.tile([D, n_steps], f32)
            ct_sb = cpool.tile([1, n_steps], f32)

            # ---- HAM warmup: back-to-back matmuls on garbage data open the
            # PE clock gate while the input DMAs land
            wsrc = cpool.tile([128, N], bf16)
            nc.vector.memset(wsrc[:], 0.0)
            pwarm = ppw.tile([D, N], f32, tag="warm")
            for _ in range(N_WARM):
                nc.tensor.matmul(
                    pwarm[:],
                    lhsT=wsrc[:, 0:D],
                    rhs=wsrc[:],
                    start=True,
                    stop=True,
                )

            # state: vTr fp32r (tanh output; 12-bit state rounding is benign
            # since W goes through the exact split W_r + W_c)
            vTr = vpool.tile([D, N], mdt, tag="vr")
            nc.sync.dma_start(vTr[:], vT0[:, :])

            for t in range(n_steps):
                for l in range(3):
                    # [v@W_r | v@W_c] chunked [128, 64] x4 packed into [128, 256]
                    pt = ppt.tile([128, 4 * 2 * D], f32, tag="pt")
                    for j in range(KC):
                        nc.tensor.matmul(
                            pt[:, 64 * j : 64 * (j + 1)],
                            lhsT=vTr[:, 128 * j : 128 * (j + 1)],
                            rhs=wg_sb[:, 2 * D * l : 2 * D * (l + 1)],
                            start=True,
                            stop=True,
                        )
                    # PE would idle during the DVE pair-reduce: fillers keep
                    # the HAM clock gate open (in-order engine stream, so
                    # these run right here)
                    for _ in range(N_FILL_W):
                        nc.tensor.matmul(
                            pwarm[:, 0:NH],
                            lhsT=vTr[:, 0:D],
                            rhs=at_sb[0:D, 0:NH],
                            start=True,
                            stop=True,
                        )
                    # t = v@W_r + v@W_c: reduce over the size-2 axis
                    ts_ = tpool.tile([128, 128], mdt, tag="ts")
                    ptv = pt[:].rearrange("p (j t f) -> p j f t", t=2, f=D)
                    tsv = ts_[:].rearrange("p (j f) -> p j f", f=D)
                    with nc.allow_low_precision(reason="2-elem pair sum to fp32r"):
                        nc.vector.reduce_sum(tsv, ptv, axis=AX.X)
                    # u^T = (Ahat t)^T in two column halves so tanh(half 0)
                    # overlaps the PE streaming half 1
                    pu = ppu.tile([D, N], f32, tag="pu")
                    for h in range(2):
                        for j in range(KC):
                            nc.tensor.matmul(
                                pu[:, NH * h : NH * (h + 1)],
                                lhsT=ts_[:, 32 * j : 32 * (j + 1)],
                                rhs=at_sb[:, N * j + NH * h : N * j + NH * (h + 1)],
                                start=(j == 0),
                                stop=(j == KC - 1),
                            )
                    # fillers covering the tanh window
                    for _ in range(N_FILL_A):
                        nc.tensor.matmul(
                            pwarm[:, 0:NH],
                            lhsT=ts_[:, 0:D],
                            rhs=at_sb[:, 0:NH],
                            start=True,
                            stop=True,
                        )
                    vTr = vpool.tile([D, N], mdt, tag="vr")
                    for h in range(2):
                        nc.scalar.activation(
                            vTr[:, NH * h : NH * (h + 1)],
                            pu[:, NH * h : NH * (h + 1)],
                            AF.Tanh,
                            bias=bg_sb[:, l : l + 1],
                        )

                # ---- readout ----
                pp1 = ppro.tile([D, N], f32, tag="pro")
                nc.tensor.matmul(pp1[:], lhsT=wm1_sb[:], rhs=vTr[:], start=True, stop=True)
                p1s = ropool.tile([D, N], mdt, tag="p1s")
                nc.scalar.activation(p1s[:], pp1[:], AF.Relu, bias=bm1_sb[:, 0:1])
                ppr = ppro.tile([1, N], f32, tag="pro")
                nc.tensor.matmul(ppr[:], lhsT=wm2_sb[:], rhs=p1s[:], start=True, stop=True)
                m = ropool.tile([1, 1], f32, tag="m")
                nc.vector.reduce_max(m[:], ppr[:], axis=AX.X)
                oh = ropool.tile([1, N], bf16, tag="oh")
                nc.vector.tensor_scalar(oh[:], ppr[:], m[:], None, op0=ALU.is_equal)
                nc.vector.reduce_sum(ct_sb[0:1, t : t + 1], oh[:], axis=AX.X)
                pob = ppro.tile([D, N], f32, tag="pro")
                nc.tensor.matmul(pob[:], lhsT=ones_b[:], rhs=oh[:], start=True, stop=True)
                scr = ropool.tile([D, N], f32, tag="scr")
                nc.vector.tensor_tensor(scr[:], vTr[:], pob[:], op=ALU.mult)
                nc.vector.reduce_sum(outT_sb[:, t : t + 1], scr[:], axis=AX.X)

            nc.sync.dma_start(outT[:, :], outT_sb[:])
            nc.sync.dma_start(ct[:, :], ct_sb[:])

    nc.compile()
    return nc


def _prepare_inputs(vertices, edge_index, W1, b1, W2, b2, W3, b3, Wm1, bm1, Wm2, bm2,
                    n_steps):
    vertices = np.asarray(vertices, np.float32)
    edge_index = np.asarray(edge_index)
    src = np.concatenate([edge_index[0].astype(np.int64), np.arange(N, dtype=np.int64)])
    dst = np.concatenate([edge_index[1].astype(np.int64), np.arange(N, dtype=np.int64)])
    deg = np.zeros(N, np.float32)
    np.add.at(deg, dst, np.float32(1.0))
    dinv = (1.0 / np.sqrt(deg)).astype(np.float32)
    A = np.zeros((N, N), np.float32)
    np.add.at(A, (dst, src), dinv[src] * dinv[dst])
    # at[k, 512*j + n] = A[n, 128*j + k]
    atT = np.ascontiguousarray(
        A.T.reshape(KC, 128, N).transpose(1, 0, 2).reshape(128, KC * N)
    )

    def round12(x):
        # fp32r: round-to-nearest 12-bit mantissa (HW-verified)
        m, e = np.frexp(np.asarray(x, np.float32))
        return np.ldexp(
            (np.round(m.astype(np.float64) * 4096.0) / 4096.0), e
        ).astype(np.float32)

    blocks = []
    for w in (W1, W2, W3):
        w = np.asarray(w, np.float32)
        wr = round12(w)
        blocks += [wr, w - wr]
    wg = np.ascontiguousarray(np.concatenate(blocks, axis=1))
    bg = np.ascontiguousarray(
        np.stack([np.asarray(b, np.float32) for b in (b1, b2, b3)], axis=1)
    )
    return {
        "atT": atT,
        "vT0": np.ascontiguousarray(vertices.T),
        "wg": wg,
        "bg": bg,
        "wm1": np.ascontiguousarray(np.asarray(Wm1, np.float32)),
        "bm1": np.ascontiguousarray(np.asarray(bm1, np.float32).reshape(D, 1)),
        "wm2": np.ascontiguousarray(np.asarray(Wm2, np.float32).reshape(D, 1)),
        "ones": np.ones((1, D), np.float32),
    }


def run(inputs, n_steps=N_STEPS, mm_dt=MM_DT, trace=False):
    """Run the bass kernel; returns (out [n_steps, 32] float32, BassKernelResults)."""
    from concourse.bass_utils import run_bass_kernel_spmd

    key = (n_steps, mm_dt)
    if key not in _CACHE:
        _CACHE[key] = _build(n_steps, mm_dt)
    nc = _CACHE[key]

    in_map = _prepare_inputs(**inputs, n_steps=n_steps)
    res = run_bass_kernel_spmd(
        nc, [dict(in_map) for _ in range(8)], core_ids=list(range(8)), trace=trace
    )
    r = res.results[0]
    out = (r["outT"] / r["ct"]).T.astype(np.float32)
    return np.ascontiguousarray(out), res


def kernel(**inputs):
    out, _ = run(inputs, n_steps=N, mm_dt=MM_DT, trace=False)
    return out
